# revision 24
# baseline (speedup 1.0000x reference)
"""Trainium2 Bass kernel for nn_Block_48610439856264 (DiT-style transformer block).

B=4, N=2048, C=512, H=8 heads, D=64, d_ff=2048, fp32 I/O.

Sharding: 8 cores = 4 batches x 2 token-halves. Each core receives the full
batch (own token half first) so k/v/s are computed locally over all 2048
tokens; q/gate/proj/MLP/output cover only the core's own 1024 tokens.

All matmuls run in fp8e4m3 DoubleRow perf mode (2 k-tiles per pass, 0.5
cycles/output-row = 4x bf16 FLOPs). Scores use a K=32x2 d-split layout
(kkT2/qT2 [32g:32g+32, 2, *]) enabled by a host-side permutation of the
q/k weight columns: head h lives at partition group g=h%4 of tile j=h//4,
with its two 32-channel halves as the DoubleRow k-tile planes.

Per-tensor power-of-2 scales keep every fp8 operand in e4m3's normal
range; compensations are folded into the exp scale (1/256), gelu scale
(1/16), and the two residual adds (1/1024, 1/16).
"""

import numpy as np
import ml_dtypes

N_CORES = 8
B, N, C = 4, 2048, 512
H, D = 8, 64
DFF = 4 * C
P = 128
NT = N // P          # 16 full-token tiles
NTO = NT // 2        # 8 own-token tiles
CT = C // P          # 4 channel tiles
DFT = DFF // P       # 16 d_ff tiles
TOK_OWN = N // 2     # 1024
EPS = 1e-5
# v_aug plane width: lhsT spans 128 cols from any head offset 65h; the
# DoubleRow LDWEIGHTS plane stride must be a multiple of 128 bytes.
VPAD = 768

# fp8 scale plan (powers of 2)
SQ = 16.0    # w_q scale (on top of D**-0.5)
SK = 16.0    # w_k, w_s scale
SV = 2.0     # w_v scale
SG = 32.0    # w_gate scale
SP = 16.0    # w_proj scale
SF1 = 16.0   # w_fc1 scale
SF2 = 16.0   # w_fc2 scale
EXP_SCALE = 1.0 / (SQ * SK)          # exp(score' * EXP_SCALE)
GELU_SCALE = 1.0 / SF1
PROJ_COMP = 1.0 / (SV * SG * SP)     # proj residual compensation
FC2_COMP = 1.0 / SF2

_CACHE = {}


def _qk_perm():
    """Column permutation for q/k/s weights: head h -> partition group h%4
    of tile j=h//4; d-half f -> DoubleRow plane f (PSUM tile m=2j+f)."""
    perm = np.empty(C, dtype=np.int64)  # perm[new] = old
    for m in range(4):
        j, f = m // 2, m % 2
        for g in range(4):
            h = 4 * j + g
            for l in range(32):
                perm[128 * m + 32 * g + l] = 64 * h + 32 * f + l
    return perm


def _enable_ldw_opt():
    """Walrus ships with --enable-ldw-opt=false; consecutive matmuls sharing
    a stationary operand then reload weights every time.  Flip the flag
    (opt-in via BASS_LDW_OPT=1) by rewriting the walrus argv."""
    import os
    if os.environ.get("BASS_LDW_OPT") != "1":
        return
    import concourse.bass_utils as bu
    if getattr(bu, "_ldw_patched", False):
        return
    orig = bu.run_command

    def run_command(cmd, *a, **kw):
        cmd = [c.replace("--enable-ldw-opt=false", "--enable-ldw-opt=true")
               if isinstance(c, str) else c for c in cmd]
        return orig(cmd, *a, **kw)

    bu.run_command = run_command
    bu._ldw_patched = True


def _build_nc():
    import concourse.bacc as bacc
    import concourse.mybir as mybir
    import concourse.tile as tile

    FP32 = mybir.dt.float32
    FP8 = mybir.dt.float8e4

    nc = bacc.Bacc("TRN2", num_devices=N_CORES)

    xb_d = nc.dram_tensor("xb", [N, C], FP32, kind="ExternalInput").ap()
    eb_d = nc.dram_tensor("eb", [N, C], FP32, kind="ExternalInput").ap()
    wqkv_d = nc.dram_tensor("wqkv", [C, 3 * C], FP8, kind="ExternalInput").ap()
    ws_d = nc.dram_tensor("ws", [C, C], FP8, kind="ExternalInput").ap()
    wgate_d = nc.dram_tensor("wgate", [C, C], FP8, kind="ExternalInput").ap()
    wproj_d = nc.dram_tensor("wproj", [C, C], FP8, kind="ExternalInput").ap()
    wfc1_d = nc.dram_tensor("wfc1", [C, DFF], FP8, kind="ExternalInput").ap()
    wfc2_d = nc.dram_tensor("wfc2", [DFF, C], FP8, kind="ExternalInput").ap()
    out_d = nc.dram_tensor("out", [TOK_OWN, C], FP32, kind="ExternalOutput").ap()

    with tile.TileContext(nc) as tc:
        _build_body(nc, tc, mybir,
                    xb_d, eb_d, out_d,
                    wqkv_d, ws_d, wgate_d, wproj_d, wfc1_d, wfc2_d)

    nc.compile()
    return nc


def _build_body(nc, tc, mybir,
                xb_r_, eb_r_, out_d,
                wqkv_d, ws_d, wgate_d, wproj_d, wfc1_d, wfc2_d):
    from contextlib import ExitStack
    from collections import deque
    from concourse.masks import make_identity

    FP32 = mybir.dt.float32
    BF16 = mybir.dt.bfloat16
    FP8 = mybir.dt.float8e4
    Act = mybir.ActivationFunctionType
    Alu = mybir.AluOpType
    DR = mybir.MatmulPerfMode.DoubleRow

    xb_r = xb_r_.rearrange("(t p) c -> t p c", p=P)
    eb_r = eb_r_.rearrange("(t p) c -> t p c", p=P)
    out_r = out_d.rearrange("(t p) c -> t p c", p=P)

    L0 = ExitStack()
    with L0:
        consts = L0.enter_context(tc.tile_pool(name="consts", bufs=1))
        stat_pool = L0.enter_context(tc.tile_pool(name="stats", bufs=6))
        z_pool = L0.enter_context(tc.tile_pool(name="zp", bufs=4))
        xo_pool = L0.enter_context(tc.tile_pool(name="xo", bufs=NTO))

        zxT_p = L0.enter_context(tc.tile_pool(name="zxTp", bufs=1))
        zxT = zxT_p.tile([P, CT, N], FP8, name="zxT")
        kk_p = L0.enter_context(tc.tile_pool(name="kkp", bufs=1))
        # per-tile 2 heads at partition bases {0, 32} (base 96 is illegal)
        kkT2 = [kk_p.tile([D, 2, N], FP8, name=f"kkT2_{j}") for j in range(4)]
        qT2 = [kk_p.tile([D, 2, TOK_OWN], FP8, name=f"qT2_{j}") for j in range(4)]
        gTh = L0.enter_context(tc.tile_pool(name="gTh", bufs=1)) \
            .tile([P, H // 2, TOK_OWN], FP8, name="gTht")
        vpool = L0.enter_context(tc.tile_pool(name="vp", bufs=NTO))
        v2 = [vpool.tile([P, 2, VPAD], FP8, name=f"v2_{i}", tag="v2")
              for i in range(NTO)]
        ogT_p = L0.enter_context(tc.tile_pool(name="ogTp", bufs=1))
        ogT = ogT_p.tile([P, CT, TOK_OWN], FP8, name="ogT")
        wA = L0.enter_context(tc.tile_pool(name="wA", bufs=1))

        # scores PSUM on the right side
        psS = L0.enter_context(
            tc.tile_pool(name="psS", bufs=2, space="PSUM", side="right"))
        epool = L0.enter_context(tc.tile_pool(name="epool", bufs=20, side="right"))
        rpool = L0.enter_context(tc.tile_pool(name="rpool", bufs=4, side="right"))

        eps_sb = consts.tile([P, 1], FP32)
        nc.vector.memset(eps_sb[:], EPS)
        ident = consts.tile([P, P], BF16)
        make_identity(nc, ident[:])

        xm_pool = L0.enter_context(tc.tile_pool(name="xm", bufs=NTO))

        x_own = [xo_pool.tile([P, C], FP32, name=f"xown{t}", tag="xown")
                 for t in range(NTO)]

        # ---------- LN helper ----------
        def ln_group(tiles, mvtag, zT_all, t0, pspool, pstag, eng):
            """LN 4 tiles token-major, transpose to c-major fp8 planes of
            zT_all at column t*128.  eng: 'a' routes the PSUM->SBUF copies
            to ACT (idle during the front), else DVE."""
            g = len(tiles)
            mv = stat_pool.tile([P, g, 2], FP32, name=f"mv_{mvtag}", tag="mv")
            st6 = stat_pool.tile([P, 6], FP32, name=f"st6_{mvtag}", tag="st6")
            for i, xt in enumerate(tiles):
                nc.vector.bn_stats(st6[:], xt[:])
                nc.vector.bn_aggr(mv[:, i, :], st6[:])
            sd = stat_pool.tile([P, g], FP32, name=f"sd_{mvtag}", tag="sd")
            nc.scalar.activation(sd[:], mv[:, :, 1], Act.Sqrt, bias=eps_sb[:])
            rstd = stat_pool.tile([P, g], FP32, name=f"rstd_{mvtag}", tag="rstd")
            nc.vector.reciprocal_approx_fast(rstd[:], sd[:])
            for i, xt in enumerate(tiles):
                t = t0 + i
                zt = z_pool.tile([P, C], BF16, name=f"z_{mvtag}_{i}", tag="z")
                nc.vector.tensor_scalar(
                    zt[:], xt[:], mv[:, i, 0:1], rstd[:, i : i + 1],
                    Alu.subtract, Alu.mult,
                )
                pt = pspool.tile([P, CT, P], BF16, name=f"pt_{mvtag}", tag=pstag)
                for c in range(CT):
                    nc.tensor.transpose(
                        pt[:, c, :], zt[:, c * P : (c + 1) * P], ident[:])
                if eng == "a":
                    nc.scalar.copy(zT_all[:, :, t * P : (t + 1) * P], pt[:])
                else:
                    nc.vector.tensor_copy(
                        zT_all[:, :, t * P : (t + 1) * P], pt[:])

        # ---------- projection helpers (DR fp8) ----------
        def q_proj(m):
            pq = psC.tile([P, TOK_OWN], FP32, name=f"pq{m}", tag="pc")
            for i in range(2):
                lw = wqkv_sb[:, 2 * i : 2 * i + 2, m * P : (m + 1) * P]
                for ch in range(2):
                    nc.tensor.matmul(
                        pq[:, ch * 512 : (ch + 1) * 512], lw,
                        zxT[:, 2 * i : 2 * i + 2, ch * 512 : (ch + 1) * 512],
                        start=(i == 0), stop=(i == 1), perf_mode=DR,
                    )
            nc.vector.tensor_copy(qT2[2 * (m // 2)][:, m % 2, :], pq[0:D, :])
            nc.vector.tensor_copy(qT2[2 * (m // 2) + 1][:, m % 2, :],
                                  pq[D : 2 * D, :])

        def kk_half(m, half):
            pc = psC.tile([P, TOK_OWN], FP32, name=f"pc{m}_{half}", tag="pc")
            base = half * TOK_OWN
            for i in range(2):
                lw = wqkv_sb[:, 2 * i : 2 * i + 2, C + m * P : C + (m + 1) * P]
                for ch in range(2):
                    sl = slice(base + ch * 512, base + (ch + 1) * 512)
                    nc.tensor.matmul(
                        pc[:, ch * 512 : (ch + 1) * 512], lw,
                        zxT[:, 2 * i : 2 * i + 2, sl],
                        start=(i == 0), stop=False, perf_mode=DR,
                    )
            for i in range(2):
                lw = ws_sb[:, 2 * i : 2 * i + 2, m * P : (m + 1) * P]
                for ch in range(2):
                    sl = slice(base + ch * 512, base + (ch + 1) * 512)
                    nc.tensor.matmul(
                        pc[:, ch * 512 : (ch + 1) * 512], lw,
                        zeT[:, 2 * i : 2 * i + 2, sl],
                        start=False, stop=(i == 1), perf_mode=DR,
                    )
            nc.vector.tensor_copy(
                kkT2[2 * (m // 2)][:, m % 2, base : base + TOK_OWN], pc[0:D, :])
            nc.vector.tensor_copy(
                kkT2[2 * (m // 2) + 1][:, m % 2, base : base + TOK_OWN],
                pc[D : 2 * D, :])

        def v_pair(tp):
            pv = psC.tile([P, TOK_OWN], FP32, name=f"pv{tp}", tag="pc")
            for i in range(2):
                for tt in range(2):
                    t = 2 * tp + tt
                    nc.tensor.matmul(
                        pv[:, tt * 512 : (tt + 1) * 512],
                        zxT[:, 2 * i : 2 * i + 2, t * P : (t + 1) * P],
                        wqkv_sb[:, 2 * i : 2 * i + 2, 2 * C : 3 * C],
                        start=(i == 0), stop=(i == 1), perf_mode=DR,
                    )
            nc.vector.memset(v2[tp][:, :, H * (D + 1) : VPAD], 0.0)
            va = v2[tp][:, :, 0 : H * (D + 1)].rearrange(
                "p two (h x) -> p two h x", x=D + 1)
            nc.vector.memset(va[:, :, :, D : D + 1], 1.0)
            for tt in range(2):
                nc.vector.tensor_copy(
                    va[:, tt, :, 0:D],
                    pv[:, tt * 512 : (tt + 1) * 512].rearrange(
                        "p (h d) -> p h d", d=D),
                )

        def gate_proj(m):
            pg = psC.tile([P, TOK_OWN], FP32, name=f"pg{m}", tag="pc")
            for i in range(2):
                lw = wgate_sb[:, 2 * i : 2 * i + 2, m * P : (m + 1) * P]
                for ch in range(2):
                    nc.tensor.matmul(
                        pg[:, ch * 512 : (ch + 1) * 512], lw,
                        zxT[:, 2 * i : 2 * i + 2, ch * 512 : (ch + 1) * 512],
                        start=(i == 0), stop=(i == 1), perf_mode=DR,
                    )
            nc.vector.tensor_copy(gTh[:, m, :], pg[:])

        # ---------- attention helpers ----------
        ep_tiles = {}  # (h, tp) -> Ep tile

        def scores_exp(h, t):
            jj, g2 = h // 2, h % 2
            sc = psS.tile([P, TOK_OWN], FP32, name=f"sc{h}_{t}", tag="sc")
            for ch in range(2):
                nc.tensor.matmul(
                    sc[:, ch * 512 : (ch + 1) * 512],
                    kkT2[jj][32 * g2 : 32 * g2 + 32, :, t * P : (t + 1) * P],
                    qT2[jj][32 * g2 : 32 * g2 + 32, :, ch * 512 : (ch + 1) * 512],
                    start=True, stop=True, perf_mode=DR,
                )
            tp = t // 2
            if t % 2 == 0:
                ep_tiles[(h, tp)] = epool.tile(
                    [P, 2, TOK_OWN], FP8, name=f"Ep{h}_{tp}", tag="E")
            nc.scalar.activation(
                ep_tiles[(h, tp)][:, t % 2, :], sc[:], Act.Exp, scale=EXP_SCALE)

        ps_att = {}

        def attnv(h, tp):
            if tp == 0:
                ps_att[h] = psO.tile([P, TOK_OWN], FP32, name=f"pso{h}", tag="po")
            ep = ep_tiles.pop((h, tp))
            for ch in range(2):
                nc.tensor.matmul(
                    ps_att[h][:, ch * 512 : (ch + 1) * 512],
                    v2[tp][:, :, h * (D + 1) : h * (D + 1) + P],
                    ep[:, :, ch * 512 : (ch + 1) * 512],
                    start=(tp == 0), stop=(tp == NTO - 1), perf_mode=DR,
                )
            if tp == NTO - 1:
                normalize(h)

        def normalize(h):
            ps_o = ps_att.pop(h)
            dn = rpool.tile([1, TOK_OWN], FP32, name="dn", tag="nrm")
            nc.vector.tensor_copy(dn[:], ps_o[D : D + 1, :])
            rdb1 = rpool.tile([1, TOK_OWN], FP32, name="rdb1", tag="nrm")
            nc.vector.reciprocal_approx_fast(rdb1[:], dn[:])
            dnb = rpool.tile([D, TOK_OWN], FP32, name="dnb", tag="nrm")
            nc.gpsimd.partition_broadcast(dnb[:], rdb1[:])
            t1 = rpool.tile([D, TOK_OWN], FP32, name="t1", tag="nrm")
            gpo = (h % 2) * D
            nc.vector.tensor_mul(t1[:], ps_o[0:D, :],
                                 gTh[gpo : gpo + D, h // 2, :])
            po = (h % 2) * D
            nc.gpsimd.tensor_mul(ogT[po : po + D, h // 2, :], t1[:], dnb[:])

        # ================= emission =================
        es_early = ExitStack()
        L2 = ExitStack()
        try:
            xr_pool = es_early.enter_context(tc.tile_pool(name="xrp", bufs=NTO))
            e_pool = es_early.enter_context(tc.tile_pool(name="ep", bufs=5))
            zeT_pool = es_early.enter_context(tc.tile_pool(name="zeTp", bufs=1))
            zeT = zeT_pool.tile([P, CT, N], FP8, name="zeT")
            wsp = es_early.enter_context(tc.tile_pool(name="wsp", bufs=1))

            psC = L2.enter_context(tc.tile_pool(name="psC", bufs=2, space="PSUM"))

            # ---- DMA in: weights + x/e half 0 first ----
            wqkv_sb = wA.tile([P, CT, 3 * C], FP8)
            nc.sync.dma_start(wqkv_sb[:], wqkv_d.rearrange("(k p) n -> p k n", p=P))
            ws_sb = wsp.tile([P, CT, C], FP8)
            nc.sync.dma_start(ws_sb[:], ws_d.rearrange("(k p) n -> p k n", p=P))

            x_all = list(x_own)
            e_all = []
            for t in range(NTO):
                nc.sync.dma_start(x_all[t][:], xb_r[t])
            for t in range(NTO):
                et = e_pool.tile([P, C], FP32, name=f"e{t}", tag="e")
                nc.sync.dma_start(et[:], eb_r[t])
                e_all.append(et)
            wgate_sb = wA.tile([P, CT, C], FP8)
            nc.sync.dma_start(wgate_sb[:], wgate_d.rearrange("(k p) n -> p k n", p=P))
            for t in range(NTO, NT):
                xt = xr_pool.tile([P, C], FP32, name=f"xr{t}", tag="xr")
                nc.sync.dma_start(xt[:], xb_r[t])
                x_all.append(xt)
            for t in range(NTO, NT):
                et = e_pool.tile([P, C], FP32, name=f"e{t}", tag="e")
                nc.sync.dma_start(et[:], eb_r[t])
                e_all.append(et)

            # ---- LN half 0 + early projections ----
            for g in range(2):
                ln_group(x_all[4 * g : 4 * g + 4], f"x{g}", zxT, 4 * g,
                         psC, "pc", "a")
                ln_group(e_all[4 * g : 4 * g + 4], f"e{g}", zeT, 4 * g,
                         psC, "pc", "a")
            q_proj(0)
            kk_half(0, 0)
            q_proj(1)
            kk_half(1, 0)

            # ---- prime: heads 0-3 x t 0-7 interleaved with the rest ----
            bg = deque()
            bg.append(lambda: ln_group(x_all[8:12], "x2", zxT, 8, psC, "pc", "v"))
            bg.append(lambda: ln_group(e_all[8:12], "e2", zeT, 8, psC, "pc", "v"))
            bg.append(lambda: ln_group(x_all[12:16], "x3", zxT, 12, psC, "pc", "v"))
            bg.append(lambda: ln_group(e_all[12:16], "e3", zeT, 12, psC, "pc", "v"))
            bg.append(lambda: kk_half(0, 1))
            bg.append(lambda: kk_half(1, 1))
            bg.append(lambda: v_pair(0))
            bg.append(lambda: v_pair(1))
            bg.append(lambda: v_pair(2))
            bg.append(lambda: v_pair(3))
            bg.append(lambda: q_proj(2))
            bg.append(lambda: q_proj(3))
            bg.append(lambda: kk_half(2, 0))
            bg.append(lambda: kk_half(3, 0))
            bg.append(lambda: kk_half(2, 1))
            bg.append(lambda: kk_half(3, 1))
            bg.append(lambda: v_pair(4))
            bg.append(lambda: v_pair(5))
            bg.append(lambda: v_pair(6))
            bg.append(lambda: v_pair(7))
            bg.append(lambda: gate_proj(0))
            bg.append(lambda: gate_proj(1))
            bg.append(lambda: gate_proj(2))
            bg.append(lambda: gate_proj(3))

            prime = [(h, t) for h in range(2) for t in range(8)]
            prime += [(h, t) for h in range(2, 4) for t in range(4)]
            for i, (h, t) in enumerate(prime):
                if bg:
                    bg.popleft()()
                scores_exp(h, t)
            while bg:
                bg.popleft()()

            es_early.close()  # xr, e tiles, zeT, ws freed
        finally:
            L2.close()  # psC banks freed before psO opens

        # MLP weights: DMA early so the tail never waits on HBM
        wE = L0.enter_context(tc.tile_pool(name="wE", bufs=1))
        wproj_sb = wE.tile([P, CT, C], FP8)
        nc.sync.dma_start(wproj_sb[:],
                          wproj_d.rearrange("(k p) n -> p k n", p=P))
        wfc1_sb = wE.tile([P, CT, DFF], FP8)
        nc.sync.dma_start(wfc1_sb[:],
                          wfc1_d.rearrange("(k p) n -> p k n", p=P))
        wfc2_sb = wE.tile([P, DFT, C], FP8)
        nc.sync.dma_start(wfc2_sb[:],
                          wfc2_d.rearrange("(k p) n -> p k n", p=P))

        # ---- steady: remaining exps + head-sequential attnv ----
        with tc.tile_pool(name="psO", bufs=2, space="PSUM") as psO:
            steady = [(h, t) for h in range(2) for t in range(8, 16)]
            steady += [(h, t) for h in range(2, 4) for t in range(4, 16)]
            steady += [(h, t) for h in range(4, 8) for t in range(16)]
            done_exp = {(h, tp): True for h in range(2) for tp in range(4)}
            done_exp.update({(h, tp): True for h in range(2, 4) for tp in range(2)})

            # pop attnv strictly in (head, tp) lexicographic order
            attnv_order = [(h, tp) for h in range(H) for tp in range(NTO)]
            cursor = 0

            def pump(budget):
                nonlocal cursor
                n = 0
                while n < budget and cursor < len(attnv_order):
                    item = attnv_order[cursor]
                    if not done_exp.get(item):
                        break
                    attnv(*item)
                    cursor += 1
                    n += 1

            lag = 8  # exp chunks of headroom before attnv consumes
            for i, (h, t) in enumerate(steady):
                # emit ready attnv work BEFORE the (possibly psS-stalled)
                # scores: the PE executes its queue in order, so a stalled
                # matmul would block ready work queued behind it
                if i >= lag:
                    pump(1)
                scores_exp(h, t)
                if t % 2 == 1:
                    done_exp[(h, t // 2)] = True
            pump(len(attnv_order))

            # ---------- proj + residual + LN3 + MLP ----------
            with (
                tc.tile_pool(name="z3Tp", bufs=1) as z3T_pool,
                tc.tile_pool(name="hTp", bufs=1) as hT_pool,
                tc.tile_pool(name="opool", bufs=4) as opool,
            ):
                xm = []

                def proj_pair(tpair):
                    pp = psO.tile([P, TOK_OWN], FP32, name=f"pp{tpair}", tag="po")
                    for tt in range(2):
                        t = 2 * tpair + tt
                        for i in range(2):
                            nc.tensor.matmul(
                                pp[:, tt * 512 : (tt + 1) * 512],
                                ogT[:, 2 * i : 2 * i + 2, t * P : (t + 1) * P],
                                wproj_sb[:, 2 * i : 2 * i + 2, :],
                                start=(i == 0), stop=(i == 1), perf_mode=DR,
                            )
                    for tt in range(2):
                        t = 2 * tpair + tt
                        xmt = xm_pool.tile([P, C], FP32, name=f"xm{t}", tag="xm")
                        nc.vector.scalar_tensor_tensor(
                            xmt[:], pp[:, tt * 512 : (tt + 1) * 512],
                            PROJ_COMP, x_own[t][:],
                            Alu.mult, Alu.add,
                        )
                        xm.append(xmt)

                z3T = z3T_pool.tile([P, CT, TOK_OWN], FP8, name="z3T")
                hT = hT_pool.tile([P, DFT, TOK_OWN], FP8, name="hT")
                for ch in range(2):
                    proj_pair(2 * ch)
                    proj_pair(2 * ch + 1)
                    ln_group(xm[4 * ch : 4 * ch + 4], f"x3{ch}", z3T, 4 * ch,
                             psS, "sc", "v")
                    sl = slice(ch * 512, (ch + 1) * 512)
                    pf2w = [psO.tile([P, TOK_OWN], FP32, name=f"pf2w{ch}{i}",
                                     tag="po") for i in range(2)]
                    for mm in range(DFT):
                        pf = psS.tile([P, 512], FP32, name=f"pf{ch}", tag="sc")
                        for i in range(2):
                            nc.tensor.matmul(
                                pf[:],
                                wfc1_sb[:, 2 * i : 2 * i + 2,
                                        mm * P : (mm + 1) * P],
                                z3T[:, 2 * i : 2 * i + 2, sl],
                                start=(i == 0), stop=(i == 1), perf_mode=DR,
                            )
                        nc.scalar.activation(hT[:, mm, sl], pf[:], Act.Gelu,
                                             scale=GELU_SCALE)
                        if mm % 2 == 1:
                            i = mm // 2
                            for tt in range(4):
                                t = 4 * ch + tt
                                nc.tensor.matmul(
                                    pf2w[tt // 2][:, (tt % 2) * 512
                                                  : (tt % 2 + 1) * 512],
                                    hT[:, 2 * i : 2 * i + 2,
                                       t * P : (t + 1) * P],
                                    wfc2_sb[:, 2 * i : 2 * i + 2, :],
                                    start=(i == 0), stop=(i == DFT // 2 - 1),
                                    perf_mode=DR,
                                )
                    for tt in range(4):
                        t = 4 * ch + tt
                        ot = opool.tile([P, C], FP32, name="ot", tag="ot")
                        nc.vector.scalar_tensor_tensor(
                            ot[:], pf2w[tt // 2][:, (tt % 2) * 512
                                                 : (tt % 2 + 1) * 512],
                            FC2_COMP, xm[t][:],
                            Alu.mult, Alu.add,
                        )
                        nc.sync.dma_start(out_r[t], ot[:])


def _preprocess(inputs):
    """Fold LN affine + attention scale + fp8 scaling into weights, apply
    the q/k column permutation (host-side, weight-only)."""
    f32 = np.float32
    ln1_w, ln1_b = f32(inputs["ln1_w"]), f32(inputs["ln1_b"])
    ln2_b = f32(inputs["ln2_b"])
    ln3_b = f32(inputs["ln3_b"])
    ln2_w = f32(inputs["ln2_w"])
    ln3_w = f32(inputs["ln3_w"])
    w_qkv = f32(inputs["w_qkv"]).copy()
    w_s = f32(inputs["w_s"])
    w_gate = f32(inputs["w_gate"])
    w_proj = f32(inputs["w_proj"])
    w_fc1 = f32(inputs["w_fc1"])
    w_fc2 = f32(inputs["w_fc2"])

    scale = D ** -0.5
    wqkv_eff = ln1_w[:, None] * w_qkv
    wqkv_eff[:, 0:C] *= scale * SQ
    wqkv_eff[:, C : 2 * C] *= SK
    wqkv_eff[:, 2 * C : 3 * C] *= SV
    b_qkv = ln1_b @ w_qkv
    ws_eff = ln2_w[:, None] * w_s * SK
    b_s = ln2_b @ w_s
    wgate_eff = ln1_w[:, None] * w_gate * SG
    b_gate = ln1_b @ w_gate
    wfc1_eff = ln3_w[:, None] * w_fc1 * SF1
    b_fc1 = ln3_b @ w_fc1 + f32(inputs["b_fc1"])
    wfc2_eff = w_fc2 * SF2

    for name, bias in [
        ("b_qkv", b_qkv), ("b_s", b_s), ("b_gate", b_gate), ("b_fc1", b_fc1),
        ("b_proj", f32(inputs["b_proj"])), ("b_fc2", f32(inputs["b_fc2"])),
    ]:
        assert np.all(bias == 0.0), f"nonzero bias {name} unsupported"

    perm = _qk_perm()
    wqkv_eff[:, 0:C] = wqkv_eff[:, perm]
    wqkv_eff[:, C : 2 * C] = wqkv_eff[:, C + perm]
    ws_eff = ws_eff[:, perm]

    f8 = ml_dtypes.float8_e4m3fn
    return {
        "wqkv": np.ascontiguousarray(wqkv_eff, dtype=f8),
        "ws": np.ascontiguousarray(ws_eff, dtype=f8),
        "wgate": np.ascontiguousarray(wgate_eff, dtype=f8),
        "wproj": np.ascontiguousarray(w_proj * SP, dtype=f8),
        "wfc1": np.ascontiguousarray(wfc1_eff, dtype=f8),
        "wfc2": np.ascontiguousarray(wfc2_eff, dtype=f8),
    }


def kernel(**inputs):
    from concourse import bass_utils

    _enable_ldw_opt()
    if "nc" not in _CACHE:
        _CACHE["nc"] = _build_nc()
    nc = _CACHE["nc"]

    w = _preprocess(inputs)
    x = np.asarray(inputs["x"], dtype=np.float32)
    e = np.asarray(inputs["e"], dtype=np.float32)

    in_maps = []
    for c in range(N_CORES):
        b, half = c // 2, c % 2
        if half == 0:
            xb, eb = x[b], e[b]
        else:
            xb = np.concatenate([x[b, TOK_OWN:], x[b, :TOK_OWN]], axis=0)
            eb = np.concatenate([e[b, TOK_OWN:], e[b, :TOK_OWN]], axis=0)
        in_maps.append({
            "xb": np.ascontiguousarray(xb),
            "eb": np.ascontiguousarray(eb),
            **w,
        })

    res = bass_utils.run_bass_kernel_spmd(
        nc, in_maps, core_ids=list(range(N_CORES)),
        trace=_CACHE.get("trace", False),
    )
    _CACHE["last_result"] = res

    out = np.empty((B, N, C), dtype=np.float32)
    for c in range(N_CORES):
        b, half = c // 2, c % 2
        out[b, half * TOK_OWN : (half + 1) * TOK_OWN] = res.results[c]["out"]
    return out


# revision 25
# speedup vs baseline: 1.0151x; 1.0151x over previous
"""Trainium2 Bass kernel for nn_Block_48610439856264 (DiT-style transformer block).

B=4, N=2048, C=512, H=8 heads, D=64, d_ff=2048, fp32 I/O.

Sharding: 8 cores = 4 batches x 2 token-halves. Each core receives the full
batch (own token half first) so k/v/s are computed locally over all 2048
tokens; q/gate/proj/MLP/output cover only the core's own 1024 tokens.

All matmuls run in fp8e4m3 DoubleRow perf mode (2 k-tiles per pass, 0.5
cycles/output-row = 4x bf16 FLOPs). Scores use a K=32x2 d-split layout
(kkT2/qT2 [32g:32g+32, 2, *]) enabled by a host-side permutation of the
q/k weight columns: head h lives at partition group g=h%4 of tile j=h//4,
with its two 32-channel halves as the DoubleRow k-tile planes.

Per-tensor power-of-2 scales keep every fp8 operand in e4m3's normal
range; compensations are folded into the exp scale (1/256), gelu scale
(1/16), and the two residual adds (1/1024, 1/16).
"""

import numpy as np
import ml_dtypes

N_CORES = 8
B, N, C = 4, 2048, 512
H, D = 8, 64
DFF = 4 * C
P = 128
NT = N // P          # 16 full-token tiles
NTO = NT // 2        # 8 own-token tiles
CT = C // P          # 4 channel tiles
DFT = DFF // P       # 16 d_ff tiles
TOK_OWN = N // 2     # 1024
EPS = 1e-5
# v_aug plane width: lhsT spans 128 cols from any head offset 65h; the
# DoubleRow LDWEIGHTS plane stride must be a multiple of 128 bytes.
VPAD = 768

# fp8 scale plan (powers of 2)
SQ = 16.0    # w_q scale (on top of D**-0.5)
SK = 16.0    # w_k, w_s scale
SV = 2.0     # w_v scale
SG = 32.0    # w_gate scale
SP = 16.0    # w_proj scale
SF1 = 16.0   # w_fc1 scale
SF2 = 16.0   # w_fc2 scale
EXP_SCALE = 1.0 / (SQ * SK)          # exp(score' * EXP_SCALE)
GELU_SCALE = 1.0 / SF1
PROJ_COMP = 1.0 / (SV * SG * SP)     # proj residual compensation
FC2_COMP = 1.0 / SF2

_CACHE = {}


def _qk_perm():
    """Column permutation for q/k/s weights: head h -> partition group h%4
    of tile j=h//4; d-half f -> DoubleRow plane f (PSUM tile m=2j+f)."""
    perm = np.empty(C, dtype=np.int64)  # perm[new] = old
    for m in range(4):
        j, f = m // 2, m % 2
        for g in range(4):
            h = 4 * j + g
            for l in range(32):
                perm[128 * m + 32 * g + l] = 64 * h + 32 * f + l
    return perm


def _enable_ldw_opt():
    """Walrus ships with --enable-ldw-opt=false; consecutive matmuls sharing
    a stationary operand then reload weights every time.  Flip the flag
    (opt-in via BASS_LDW_OPT=1) by rewriting the walrus argv."""
    import os
    if os.environ.get("BASS_LDW_OPT") != "1":
        return
    import concourse.bass_utils as bu
    if getattr(bu, "_ldw_patched", False):
        return
    orig = bu.run_command

    def run_command(cmd, *a, **kw):
        cmd = [c.replace("--enable-ldw-opt=false", "--enable-ldw-opt=true")
               if isinstance(c, str) else c for c in cmd]
        return orig(cmd, *a, **kw)

    bu.run_command = run_command
    bu._ldw_patched = True


def _build_nc():
    import concourse.bacc as bacc
    import concourse.mybir as mybir
    import concourse.tile as tile

    FP32 = mybir.dt.float32
    FP8 = mybir.dt.float8e4

    nc = bacc.Bacc("TRN2", num_devices=N_CORES)

    xb_d = nc.dram_tensor("xb", [N, C], FP32, kind="ExternalInput").ap()
    eb_d = nc.dram_tensor("eb", [N, C], FP32, kind="ExternalInput").ap()
    wqkv_d = nc.dram_tensor("wqkv", [C, 3 * C], FP8, kind="ExternalInput").ap()
    ws_d = nc.dram_tensor("ws", [C, C], FP8, kind="ExternalInput").ap()
    wgate_d = nc.dram_tensor("wgate", [C, C], FP8, kind="ExternalInput").ap()
    wproj_d = nc.dram_tensor("wproj", [C, C], FP8, kind="ExternalInput").ap()
    wfc1_d = nc.dram_tensor("wfc1", [C, DFF], FP8, kind="ExternalInput").ap()
    wfc2_d = nc.dram_tensor("wfc2", [DFF, C], FP8, kind="ExternalInput").ap()
    out_d = nc.dram_tensor("out", [TOK_OWN, C], FP32, kind="ExternalOutput").ap()

    with tile.TileContext(nc) as tc:
        _build_body(nc, tc, mybir,
                    xb_d, eb_d, out_d,
                    wqkv_d, ws_d, wgate_d, wproj_d, wfc1_d, wfc2_d)

    nc.compile()
    return nc


def _build_body(nc, tc, mybir,
                xb_r_, eb_r_, out_d,
                wqkv_d, ws_d, wgate_d, wproj_d, wfc1_d, wfc2_d):
    from contextlib import ExitStack
    from collections import deque
    from concourse.masks import make_identity

    FP32 = mybir.dt.float32
    BF16 = mybir.dt.bfloat16
    FP8 = mybir.dt.float8e4
    Act = mybir.ActivationFunctionType
    Alu = mybir.AluOpType
    DR = mybir.MatmulPerfMode.DoubleRow

    xb_r = xb_r_.rearrange("(t p) c -> t p c", p=P)
    eb_r = eb_r_.rearrange("(t p) c -> t p c", p=P)
    out_r = out_d.rearrange("(t p) c -> t p c", p=P)

    L0 = ExitStack()
    with L0:
        consts = L0.enter_context(tc.tile_pool(name="consts", bufs=1))
        stat_pool = L0.enter_context(tc.tile_pool(name="stats", bufs=6))
        z_pool = L0.enter_context(tc.tile_pool(name="zp", bufs=4))
        xo_pool = L0.enter_context(tc.tile_pool(name="xo", bufs=NTO))

        zxT_p = L0.enter_context(tc.tile_pool(name="zxTp", bufs=1))
        zxT = zxT_p.tile([P, CT, N], FP8, name="zxT")
        kk_p = L0.enter_context(tc.tile_pool(name="kkp", bufs=1))
        # per-tile 2 heads at partition bases {0, 32} (base 96 is illegal)
        kkT2 = [kk_p.tile([D, 2, N], FP8, name=f"kkT2_{j}") for j in range(4)]
        qT2 = [kk_p.tile([D, 2, TOK_OWN], FP8, name=f"qT2_{j}") for j in range(4)]
        gTh = L0.enter_context(tc.tile_pool(name="gTh", bufs=1)) \
            .tile([P, H // 2, TOK_OWN], FP8, name="gTht")
        vpool = L0.enter_context(tc.tile_pool(name="vp", bufs=NTO))
        v2 = [vpool.tile([P, 2, VPAD], FP8, name=f"v2_{i}", tag="v2")
              for i in range(NTO)]
        ogT_p = L0.enter_context(tc.tile_pool(name="ogTp", bufs=1))
        ogT = ogT_p.tile([P, CT, TOK_OWN], FP8, name="ogT")
        wA = L0.enter_context(tc.tile_pool(name="wA", bufs=1))

        # scores PSUM on the right side
        psS = L0.enter_context(
            tc.tile_pool(name="psS", bufs=2, space="PSUM", side="right"))
        epool = L0.enter_context(tc.tile_pool(name="epool", bufs=20, side="right"))
        rpool = L0.enter_context(tc.tile_pool(name="rpool", bufs=4, side="right"))

        eps_sb = consts.tile([P, 1], FP32)
        nc.vector.memset(eps_sb[:], EPS)
        ident = consts.tile([P, P], BF16)
        make_identity(nc, ident[:])

        xm_pool = L0.enter_context(tc.tile_pool(name="xm", bufs=NTO))

        x_own = [xo_pool.tile([P, C], FP32, name=f"xown{t}", tag="xown")
                 for t in range(NTO)]

        # ---------- LN helper ----------
        def ln_group(tiles, mvtag, zT_all, t0, pspool, pstag, eng):
            """LN 4 tiles token-major, transpose to c-major fp8 planes of
            zT_all at column t*128.  eng: 'a' routes the PSUM->SBUF copies
            to ACT (idle during the front), else DVE."""
            g = len(tiles)
            mv = stat_pool.tile([P, g, 2], FP32, name=f"mv_{mvtag}", tag="mv")
            st6 = stat_pool.tile([P, 6], FP32, name=f"st6_{mvtag}", tag="st6")
            for i, xt in enumerate(tiles):
                nc.vector.bn_stats(st6[:], xt[:])
                nc.vector.bn_aggr(mv[:, i, :], st6[:])
            sd = stat_pool.tile([P, g], FP32, name=f"sd_{mvtag}", tag="sd")
            nc.scalar.activation(sd[:], mv[:, :, 1], Act.Sqrt, bias=eps_sb[:])
            rstd = stat_pool.tile([P, g], FP32, name=f"rstd_{mvtag}", tag="rstd")
            nc.vector.reciprocal_approx_fast(rstd[:], sd[:])
            for i, xt in enumerate(tiles):
                t = t0 + i
                zt = z_pool.tile([P, C], BF16, name=f"z_{mvtag}_{i}", tag="z")
                nc.vector.tensor_scalar(
                    zt[:], xt[:], mv[:, i, 0:1], rstd[:, i : i + 1],
                    Alu.subtract, Alu.mult,
                )
                pt = pspool.tile([P, CT, P], BF16, name=f"pt_{mvtag}", tag=pstag)
                for c in range(CT):
                    nc.tensor.transpose(
                        pt[:, c, :], zt[:, c * P : (c + 1) * P], ident[:])
                if eng == "a":
                    nc.scalar.copy(zT_all[:, :, t * P : (t + 1) * P], pt[:])
                else:
                    nc.vector.tensor_copy(
                        zT_all[:, :, t * P : (t + 1) * P], pt[:])

        # ---------- projection helpers (DR fp8) ----------
        def q_proj(m):
            pq = psC.tile([P, TOK_OWN], FP32, name=f"pq{m}", tag="pc")
            for i in range(2):
                lw = wqkv_sb[:, 2 * i : 2 * i + 2, m * P : (m + 1) * P]
                for ch in range(2):
                    nc.tensor.matmul(
                        pq[:, ch * 512 : (ch + 1) * 512], lw,
                        zxT[:, 2 * i : 2 * i + 2, ch * 512 : (ch + 1) * 512],
                        start=(i == 0), stop=(i == 1), perf_mode=DR,
                    )
            nc.scalar.copy(qT2[2 * (m // 2)][:, m % 2, :], pq[0:D, :])
            nc.scalar.copy(qT2[2 * (m // 2) + 1][:, m % 2, :],
                           pq[D : 2 * D, :])

        def kk_half(m, half):
            pc = psC.tile([P, TOK_OWN], FP32, name=f"pc{m}_{half}", tag="pc")
            base = half * TOK_OWN
            for i in range(2):
                lw = wqkv_sb[:, 2 * i : 2 * i + 2, C + m * P : C + (m + 1) * P]
                for ch in range(2):
                    sl = slice(base + ch * 512, base + (ch + 1) * 512)
                    nc.tensor.matmul(
                        pc[:, ch * 512 : (ch + 1) * 512], lw,
                        zxT[:, 2 * i : 2 * i + 2, sl],
                        start=(i == 0), stop=False, perf_mode=DR,
                    )
            for i in range(2):
                lw = ws_sb[:, 2 * i : 2 * i + 2, m * P : (m + 1) * P]
                for ch in range(2):
                    sl = slice(base + ch * 512, base + (ch + 1) * 512)
                    nc.tensor.matmul(
                        pc[:, ch * 512 : (ch + 1) * 512], lw,
                        zeT[:, 2 * i : 2 * i + 2, sl],
                        start=False, stop=(i == 1), perf_mode=DR,
                    )
            nc.scalar.copy(
                kkT2[2 * (m // 2)][:, m % 2, base : base + TOK_OWN], pc[0:D, :])
            nc.scalar.copy(
                kkT2[2 * (m // 2) + 1][:, m % 2, base : base + TOK_OWN],
                pc[D : 2 * D, :])

        def v_pair(tp):
            pv = psC.tile([P, TOK_OWN], FP32, name=f"pv{tp}", tag="pc")
            for i in range(2):
                for tt in range(2):
                    t = 2 * tp + tt
                    nc.tensor.matmul(
                        pv[:, tt * 512 : (tt + 1) * 512],
                        zxT[:, 2 * i : 2 * i + 2, t * P : (t + 1) * P],
                        wqkv_sb[:, 2 * i : 2 * i + 2, 2 * C : 3 * C],
                        start=(i == 0), stop=(i == 1), perf_mode=DR,
                    )
            nc.vector.memset(v2[tp][:, :, H * (D + 1) : VPAD], 0.0)
            va = v2[tp][:, :, 0 : H * (D + 1)].rearrange(
                "p two (h x) -> p two h x", x=D + 1)
            nc.vector.memset(va[:, :, :, D : D + 1], 1.0)
            for tt in range(2):
                nc.vector.tensor_copy(
                    va[:, tt, :, 0:D],
                    pv[:, tt * 512 : (tt + 1) * 512].rearrange(
                        "p (h d) -> p h d", d=D),
                )

        def gate_proj(m):
            pg = psC.tile([P, TOK_OWN], FP32, name=f"pg{m}", tag="pc")
            for i in range(2):
                lw = wgate_sb[:, 2 * i : 2 * i + 2, m * P : (m + 1) * P]
                for ch in range(2):
                    nc.tensor.matmul(
                        pg[:, ch * 512 : (ch + 1) * 512], lw,
                        zxT[:, 2 * i : 2 * i + 2, ch * 512 : (ch + 1) * 512],
                        start=(i == 0), stop=(i == 1), perf_mode=DR,
                    )
            nc.vector.tensor_copy(gTh[:, m, :], pg[:])

        # ---------- attention helpers ----------
        ep_tiles = {}  # (h, tp) -> Ep tile

        def scores_exp(h, t):
            jj, g2 = h // 2, h % 2
            sc = psS.tile([P, TOK_OWN], FP32, name=f"sc{h}_{t}", tag="sc")
            for ch in range(2):
                nc.tensor.matmul(
                    sc[:, ch * 512 : (ch + 1) * 512],
                    kkT2[jj][32 * g2 : 32 * g2 + 32, :, t * P : (t + 1) * P],
                    qT2[jj][32 * g2 : 32 * g2 + 32, :, ch * 512 : (ch + 1) * 512],
                    start=True, stop=True, perf_mode=DR,
                )
            tp = t // 2
            if t % 2 == 0:
                ep_tiles[(h, tp)] = epool.tile(
                    [P, 2, TOK_OWN], FP8, name=f"Ep{h}_{tp}", tag="E")
            nc.scalar.activation(
                ep_tiles[(h, tp)][:, t % 2, :], sc[:], Act.Exp, scale=EXP_SCALE)

        ps_att = {}

        def attnv(h, tp):
            if tp == 0:
                ps_att[h] = psO.tile([P, TOK_OWN], FP32, name=f"pso{h}", tag="po")
            ep = ep_tiles.pop((h, tp))
            for ch in range(2):
                nc.tensor.matmul(
                    ps_att[h][:, ch * 512 : (ch + 1) * 512],
                    v2[tp][:, :, h * (D + 1) : h * (D + 1) + P],
                    ep[:, :, ch * 512 : (ch + 1) * 512],
                    start=(tp == 0), stop=(tp == NTO - 1), perf_mode=DR,
                )
            if tp == NTO - 1:
                normalize(h)

        def normalize(h):
            ps_o = ps_att.pop(h)
            dn = rpool.tile([1, TOK_OWN], FP32, name="dn", tag="nrm")
            nc.vector.tensor_copy(dn[:], ps_o[D : D + 1, :])
            rdb1 = rpool.tile([1, TOK_OWN], FP32, name="rdb1", tag="nrm")
            nc.vector.reciprocal_approx_fast(rdb1[:], dn[:])
            dnb = rpool.tile([D, TOK_OWN], FP32, name="dnb", tag="nrm")
            nc.gpsimd.partition_broadcast(dnb[:], rdb1[:])
            t1 = rpool.tile([D, TOK_OWN], FP32, name="t1", tag="nrm")
            gpo = (h % 2) * D
            nc.vector.tensor_mul(t1[:], ps_o[0:D, :],
                                 gTh[gpo : gpo + D, h // 2, :])
            po = (h % 2) * D
            nc.gpsimd.tensor_mul(ogT[po : po + D, h // 2, :], t1[:], dnb[:])

        # ================= emission =================
        es_early = ExitStack()
        L2 = ExitStack()
        try:
            xr_pool = es_early.enter_context(tc.tile_pool(name="xrp", bufs=NTO))
            e_pool = es_early.enter_context(tc.tile_pool(name="ep", bufs=5))
            zeT_pool = es_early.enter_context(tc.tile_pool(name="zeTp", bufs=1))
            zeT = zeT_pool.tile([P, CT, N], FP8, name="zeT")
            wsp = es_early.enter_context(tc.tile_pool(name="wsp", bufs=1))

            psC = L2.enter_context(tc.tile_pool(name="psC", bufs=2, space="PSUM"))

            # ---- DMA in: weights + x/e half 0 first ----
            wqkv_sb = wA.tile([P, CT, 3 * C], FP8)
            nc.sync.dma_start(wqkv_sb[:], wqkv_d.rearrange("(k p) n -> p k n", p=P))
            ws_sb = wsp.tile([P, CT, C], FP8)
            nc.sync.dma_start(ws_sb[:], ws_d.rearrange("(k p) n -> p k n", p=P))

            x_all = list(x_own)
            e_all = []
            for t in range(NTO):
                nc.sync.dma_start(x_all[t][:], xb_r[t])
            for t in range(NTO):
                et = e_pool.tile([P, C], FP32, name=f"e{t}", tag="e")
                nc.sync.dma_start(et[:], eb_r[t])
                e_all.append(et)
            wgate_sb = wA.tile([P, CT, C], FP8)
            nc.sync.dma_start(wgate_sb[:], wgate_d.rearrange("(k p) n -> p k n", p=P))
            for t in range(NTO, NT):
                xt = xr_pool.tile([P, C], FP32, name=f"xr{t}", tag="xr")
                nc.sync.dma_start(xt[:], xb_r[t])
                x_all.append(xt)
            for t in range(NTO, NT):
                et = e_pool.tile([P, C], FP32, name=f"e{t}", tag="e")
                nc.sync.dma_start(et[:], eb_r[t])
                e_all.append(et)

            # ---- LN half 0 + early projections ----
            for g in range(2):
                ln_group(x_all[4 * g : 4 * g + 4], f"x{g}", zxT, 4 * g,
                         psC, "pc", "a")
                ln_group(e_all[4 * g : 4 * g + 4], f"e{g}", zeT, 4 * g,
                         psC, "pc", "a")
            q_proj(0)
            kk_half(0, 0)
            q_proj(1)
            kk_half(1, 0)

            # ---- prime: heads 0-3 x t 0-7 interleaved with the rest ----
            bg = deque()
            bg.append(lambda: ln_group(x_all[8:12], "x2", zxT, 8, psC, "pc", "a"))
            bg.append(lambda: ln_group(e_all[8:12], "e2", zeT, 8, psC, "pc", "a"))
            bg.append(lambda: ln_group(x_all[12:16], "x3", zxT, 12, psC, "pc", "a"))
            bg.append(lambda: ln_group(e_all[12:16], "e3", zeT, 12, psC, "pc", "a"))
            bg.append(lambda: kk_half(0, 1))
            bg.append(lambda: kk_half(1, 1))
            bg.append(lambda: v_pair(0))
            bg.append(lambda: v_pair(1))
            bg.append(lambda: v_pair(2))
            bg.append(lambda: v_pair(3))
            bg.append(lambda: q_proj(2))
            bg.append(lambda: q_proj(3))
            bg.append(lambda: kk_half(2, 0))
            bg.append(lambda: kk_half(3, 0))
            bg.append(lambda: kk_half(2, 1))
            bg.append(lambda: kk_half(3, 1))
            bg.append(lambda: v_pair(4))
            bg.append(lambda: v_pair(5))
            bg.append(lambda: v_pair(6))
            bg.append(lambda: v_pair(7))
            bg.append(lambda: gate_proj(0))
            bg.append(lambda: gate_proj(1))
            bg.append(lambda: gate_proj(2))
            bg.append(lambda: gate_proj(3))

            prime = [(h, t) for h in range(2) for t in range(8)]
            prime += [(h, t) for h in range(2, 4) for t in range(4)]
            for i, (h, t) in enumerate(prime):
                if bg:
                    bg.popleft()()
                scores_exp(h, t)
            while bg:
                bg.popleft()()

            es_early.close()  # xr, e tiles, zeT, ws freed
        finally:
            L2.close()  # psC banks freed before psO opens

        # MLP weights: DMA early so the tail never waits on HBM
        wE = L0.enter_context(tc.tile_pool(name="wE", bufs=1))
        wproj_sb = wE.tile([P, CT, C], FP8)
        nc.sync.dma_start(wproj_sb[:],
                          wproj_d.rearrange("(k p) n -> p k n", p=P))
        wfc1_sb = wE.tile([P, CT, DFF], FP8)
        nc.sync.dma_start(wfc1_sb[:],
                          wfc1_d.rearrange("(k p) n -> p k n", p=P))
        wfc2_sb = wE.tile([P, DFT, C], FP8)
        nc.sync.dma_start(wfc2_sb[:],
                          wfc2_d.rearrange("(k p) n -> p k n", p=P))

        # ---- steady: remaining exps + head-sequential attnv ----
        with tc.tile_pool(name="psO", bufs=2, space="PSUM") as psO:
            steady = [(h, t) for h in range(2) for t in range(8, 16)]
            steady += [(h, t) for h in range(2, 4) for t in range(4, 16)]
            steady += [(h, t) for h in range(4, 8) for t in range(16)]
            done_exp = {(h, tp): True for h in range(2) for tp in range(4)}
            done_exp.update({(h, tp): True for h in range(2, 4) for tp in range(2)})

            # pop attnv strictly in (head, tp) lexicographic order
            attnv_order = [(h, tp) for h in range(H) for tp in range(NTO)]
            cursor = 0

            def pump(budget):
                nonlocal cursor
                n = 0
                while n < budget and cursor < len(attnv_order):
                    item = attnv_order[cursor]
                    if not done_exp.get(item):
                        break
                    attnv(*item)
                    cursor += 1
                    n += 1

            lag = 8  # exp chunks of headroom before attnv consumes
            for i, (h, t) in enumerate(steady):
                # emit ready attnv work BEFORE the (possibly psS-stalled)
                # scores: the PE executes its queue in order, so a stalled
                # matmul would block ready work queued behind it
                if i >= lag:
                    pump(1)
                scores_exp(h, t)
                if t % 2 == 1:
                    done_exp[(h, t // 2)] = True
            pump(len(attnv_order))

            # ---------- proj + residual + LN3 + MLP ----------
            with (
                tc.tile_pool(name="z3Tp", bufs=1) as z3T_pool,
                tc.tile_pool(name="hTp", bufs=1) as hT_pool,
                tc.tile_pool(name="opool", bufs=4) as opool,
            ):
                xm = []

                def proj_pair(tpair):
                    pp = psO.tile([P, TOK_OWN], FP32, name=f"pp{tpair}", tag="po")
                    for tt in range(2):
                        t = 2 * tpair + tt
                        for i in range(2):
                            nc.tensor.matmul(
                                pp[:, tt * 512 : (tt + 1) * 512],
                                ogT[:, 2 * i : 2 * i + 2, t * P : (t + 1) * P],
                                wproj_sb[:, 2 * i : 2 * i + 2, :],
                                start=(i == 0), stop=(i == 1), perf_mode=DR,
                            )
                    for tt in range(2):
                        t = 2 * tpair + tt
                        xmt = xm_pool.tile([P, C], FP32, name=f"xm{t}", tag="xm")
                        nc.vector.scalar_tensor_tensor(
                            xmt[:], pp[:, tt * 512 : (tt + 1) * 512],
                            PROJ_COMP, x_own[t][:],
                            Alu.mult, Alu.add,
                        )
                        xm.append(xmt)

                z3T = z3T_pool.tile([P, CT, TOK_OWN], FP8, name="z3T")
                hT = hT_pool.tile([P, DFT, TOK_OWN], FP8, name="hT")
                for ch in range(2):
                    proj_pair(2 * ch)
                    proj_pair(2 * ch + 1)
                    ln_group(xm[4 * ch : 4 * ch + 4], f"x3{ch}", z3T, 4 * ch,
                             psS, "sc", "v")
                    sl = slice(ch * 512, (ch + 1) * 512)
                    pf2w = [psO.tile([P, TOK_OWN], FP32, name=f"pf2w{ch}{i}",
                                     tag="po") for i in range(2)]
                    for mm in range(DFT):
                        pf = psS.tile([P, 512], FP32, name=f"pf{ch}", tag="sc")
                        for i in range(2):
                            nc.tensor.matmul(
                                pf[:],
                                wfc1_sb[:, 2 * i : 2 * i + 2,
                                        mm * P : (mm + 1) * P],
                                z3T[:, 2 * i : 2 * i + 2, sl],
                                start=(i == 0), stop=(i == 1), perf_mode=DR,
                            )
                        nc.scalar.activation(hT[:, mm, sl], pf[:], Act.Gelu,
                                             scale=GELU_SCALE)
                        if mm % 2 == 1:
                            i = mm // 2
                            for tt in range(4):
                                t = 4 * ch + tt
                                nc.tensor.matmul(
                                    pf2w[tt // 2][:, (tt % 2) * 512
                                                  : (tt % 2 + 1) * 512],
                                    hT[:, 2 * i : 2 * i + 2,
                                       t * P : (t + 1) * P],
                                    wfc2_sb[:, 2 * i : 2 * i + 2, :],
                                    start=(i == 0), stop=(i == DFT // 2 - 1),
                                    perf_mode=DR,
                                )
                    for tt in range(4):
                        t = 4 * ch + tt
                        ot = opool.tile([P, C], FP32, name="ot", tag="ot")
                        nc.vector.scalar_tensor_tensor(
                            ot[:], pf2w[tt // 2][:, (tt % 2) * 512
                                                 : (tt % 2 + 1) * 512],
                            FC2_COMP, xm[t][:],
                            Alu.mult, Alu.add,
                        )
                        nc.sync.dma_start(out_r[t], ot[:])


def _preprocess(inputs):
    """Fold LN affine + attention scale + fp8 scaling into weights, apply
    the q/k column permutation (host-side, weight-only)."""
    f32 = np.float32
    ln1_w, ln1_b = f32(inputs["ln1_w"]), f32(inputs["ln1_b"])
    ln2_b = f32(inputs["ln2_b"])
    ln3_b = f32(inputs["ln3_b"])
    ln2_w = f32(inputs["ln2_w"])
    ln3_w = f32(inputs["ln3_w"])
    w_qkv = f32(inputs["w_qkv"]).copy()
    w_s = f32(inputs["w_s"])
    w_gate = f32(inputs["w_gate"])
    w_proj = f32(inputs["w_proj"])
    w_fc1 = f32(inputs["w_fc1"])
    w_fc2 = f32(inputs["w_fc2"])

    scale = D ** -0.5
    wqkv_eff = ln1_w[:, None] * w_qkv
    wqkv_eff[:, 0:C] *= scale * SQ
    wqkv_eff[:, C : 2 * C] *= SK
    wqkv_eff[:, 2 * C : 3 * C] *= SV
    b_qkv = ln1_b @ w_qkv
    ws_eff = ln2_w[:, None] * w_s * SK
    b_s = ln2_b @ w_s
    wgate_eff = ln1_w[:, None] * w_gate * SG
    b_gate = ln1_b @ w_gate
    wfc1_eff = ln3_w[:, None] * w_fc1 * SF1
    b_fc1 = ln3_b @ w_fc1 + f32(inputs["b_fc1"])
    wfc2_eff = w_fc2 * SF2

    for name, bias in [
        ("b_qkv", b_qkv), ("b_s", b_s), ("b_gate", b_gate), ("b_fc1", b_fc1),
        ("b_proj", f32(inputs["b_proj"])), ("b_fc2", f32(inputs["b_fc2"])),
    ]:
        assert np.all(bias == 0.0), f"nonzero bias {name} unsupported"

    perm = _qk_perm()
    wqkv_eff[:, 0:C] = wqkv_eff[:, perm]
    wqkv_eff[:, C : 2 * C] = wqkv_eff[:, C + perm]
    ws_eff = ws_eff[:, perm]

    f8 = ml_dtypes.float8_e4m3fn
    return {
        "wqkv": np.ascontiguousarray(wqkv_eff, dtype=f8),
        "ws": np.ascontiguousarray(ws_eff, dtype=f8),
        "wgate": np.ascontiguousarray(wgate_eff, dtype=f8),
        "wproj": np.ascontiguousarray(w_proj * SP, dtype=f8),
        "wfc1": np.ascontiguousarray(wfc1_eff, dtype=f8),
        "wfc2": np.ascontiguousarray(wfc2_eff, dtype=f8),
    }


def kernel(**inputs):
    from concourse import bass_utils

    _enable_ldw_opt()
    if "nc" not in _CACHE:
        _CACHE["nc"] = _build_nc()
    nc = _CACHE["nc"]

    w = _preprocess(inputs)
    x = np.asarray(inputs["x"], dtype=np.float32)
    e = np.asarray(inputs["e"], dtype=np.float32)

    in_maps = []
    for c in range(N_CORES):
        b, half = c // 2, c % 2
        if half == 0:
            xb, eb = x[b], e[b]
        else:
            xb = np.concatenate([x[b, TOK_OWN:], x[b, :TOK_OWN]], axis=0)
            eb = np.concatenate([e[b, TOK_OWN:], e[b, :TOK_OWN]], axis=0)
        in_maps.append({
            "xb": np.ascontiguousarray(xb),
            "eb": np.ascontiguousarray(eb),
            **w,
        })

    res = bass_utils.run_bass_kernel_spmd(
        nc, in_maps, core_ids=list(range(N_CORES)),
        trace=_CACHE.get("trace", False),
    )
    _CACHE["last_result"] = res

    out = np.empty((B, N, C), dtype=np.float32)
    for c in range(N_CORES):
        b, half = c // 2, c % 2
        out[b, half * TOK_OWN : (half + 1) * TOK_OWN] = res.results[c]["out"]
    return out


# revision 26
# speedup vs baseline: 1.0478x; 1.0322x over previous
"""Trainium2 Bass kernel for nn_Block_48610439856264 (DiT-style transformer block).

B=4, N=2048, C=512, H=8 heads, D=64, d_ff=2048, fp32 I/O.

Sharding: 8 cores = 4 batches x 2 token-halves. Each core receives the full
batch (own token half first) so k/v/s are computed locally over all 2048
tokens; q/gate/proj/MLP/output cover only the core's own 1024 tokens.

All matmuls run in fp8e4m3 DoubleRow perf mode (2 k-tiles per pass, 0.5
cycles/output-row = 4x bf16 FLOPs). Scores use a K=32x2 d-split layout
(kkT2/qT2 [32g:32g+32, 2, *]) enabled by a host-side permutation of the
q/k weight columns: head h lives at partition group g=h%4 of tile j=h//4,
with its two 32-channel halves as the DoubleRow k-tile planes.

Per-tensor power-of-2 scales keep every fp8 operand in e4m3's normal
range; compensations are folded into the exp scale (1/256), gelu scale
(1/16), and the two residual adds (1/1024, 1/16).
"""

import numpy as np
import ml_dtypes

N_CORES = 8
B, N, C = 4, 2048, 512
H, D = 8, 64
DFF = 4 * C
P = 128
NT = N // P          # 16 full-token tiles
NTO = NT // 2        # 8 own-token tiles
CT = C // P          # 4 channel tiles
DFT = DFF // P       # 16 d_ff tiles
TOK_OWN = N // 2     # 1024
EPS = 1e-5
# v_aug plane width: lhsT spans 128 cols from any head offset 65h; the
# DoubleRow LDWEIGHTS plane stride must be a multiple of 128 bytes.
VPAD = 768

# fp8 scale plan (powers of 2)
SQ = 16.0    # w_q scale (on top of D**-0.5)
SK = 16.0    # w_k, w_s scale
SV = 2.0     # w_v scale
SG = 32.0    # w_gate scale
SP = 16.0    # w_proj scale
SF1 = 16.0   # w_fc1 scale
SF2 = 16.0   # w_fc2 scale
EXP_SCALE = 1.0 / (SQ * SK)          # exp(score' * EXP_SCALE)
GELU_SCALE = 1.0 / SF1
PROJ_COMP = 1.0 / (SV * SG * SP)     # proj residual compensation
FC2_COMP = 1.0 / SF2

_CACHE = {}


def _qk_perm():
    """Column permutation for q/k/s weights: head h -> partition group h%4
    of tile j=h//4; d-half f -> DoubleRow plane f (PSUM tile m=2j+f)."""
    perm = np.empty(C, dtype=np.int64)  # perm[new] = old
    for m in range(4):
        j, f = m // 2, m % 2
        for g in range(4):
            h = 4 * j + g
            for l in range(32):
                perm[128 * m + 32 * g + l] = 64 * h + 32 * f + l
    return perm


def _enable_ldw_opt():
    """Walrus ships with --enable-ldw-opt=false; consecutive matmuls sharing
    a stationary operand then reload weights every time.  Flip the flag
    (opt-in via BASS_LDW_OPT=1) by rewriting the walrus argv."""
    import os
    if os.environ.get("BASS_LDW_OPT") != "1":
        return
    import concourse.bass_utils as bu
    if getattr(bu, "_ldw_patched", False):
        return
    orig = bu.run_command

    def run_command(cmd, *a, **kw):
        cmd = [c.replace("--enable-ldw-opt=false", "--enable-ldw-opt=true")
               if isinstance(c, str) else c for c in cmd]
        return orig(cmd, *a, **kw)

    bu.run_command = run_command
    bu._ldw_patched = True


def _build_nc():
    import concourse.bacc as bacc
    import concourse.mybir as mybir
    import concourse.tile as tile

    FP32 = mybir.dt.float32
    FP8 = mybir.dt.float8e4

    nc = bacc.Bacc("TRN2", num_devices=N_CORES)

    xb_d = nc.dram_tensor("xb", [N, C], FP32, kind="ExternalInput").ap()
    eb_d = nc.dram_tensor("eb", [N, C], FP32, kind="ExternalInput").ap()
    wqkv_d = nc.dram_tensor("wqkv", [C, 3 * C], FP8, kind="ExternalInput").ap()
    ws_d = nc.dram_tensor("ws", [C, C], FP8, kind="ExternalInput").ap()
    wgate_d = nc.dram_tensor("wgate", [C, C], FP8, kind="ExternalInput").ap()
    wproj_d = nc.dram_tensor("wproj", [C, C], FP8, kind="ExternalInput").ap()
    wfc1_d = nc.dram_tensor("wfc1", [C, DFF], FP8, kind="ExternalInput").ap()
    wfc2_d = nc.dram_tensor("wfc2", [DFF, C], FP8, kind="ExternalInput").ap()
    out_d = nc.dram_tensor("out", [TOK_OWN, C], FP32, kind="ExternalOutput").ap()

    with tile.TileContext(nc) as tc:
        _build_body(nc, tc, mybir,
                    xb_d, eb_d, out_d,
                    wqkv_d, ws_d, wgate_d, wproj_d, wfc1_d, wfc2_d)

    nc.compile()
    return nc


def _build_body(nc, tc, mybir,
                xb_r_, eb_r_, out_d,
                wqkv_d, ws_d, wgate_d, wproj_d, wfc1_d, wfc2_d):
    from contextlib import ExitStack
    from collections import deque
    from concourse.masks import make_identity

    FP32 = mybir.dt.float32
    BF16 = mybir.dt.bfloat16
    FP8 = mybir.dt.float8e4
    Act = mybir.ActivationFunctionType
    Alu = mybir.AluOpType
    DR = mybir.MatmulPerfMode.DoubleRow

    xb_r = xb_r_.rearrange("(t p) c -> t p c", p=P)
    eb_r = eb_r_.rearrange("(t p) c -> t p c", p=P)
    out_r = out_d.rearrange("(t p) c -> t p c", p=P)

    L0 = ExitStack()
    with L0:
        consts = L0.enter_context(tc.tile_pool(name="consts", bufs=1))
        stat_pool = L0.enter_context(tc.tile_pool(name="stats", bufs=6))
        z_pool = L0.enter_context(tc.tile_pool(name="zp", bufs=4))
        xo_pool = L0.enter_context(tc.tile_pool(name="xo", bufs=NTO))

        zxT_p = L0.enter_context(tc.tile_pool(name="zxTp", bufs=1))
        zxT = zxT_p.tile([P, CT, N], FP8, name="zxT")
        kk_p = L0.enter_context(tc.tile_pool(name="kkp", bufs=1))
        # per-tile 2 heads at partition bases {0, 32} (base 96 is illegal)
        kkT2 = [kk_p.tile([D, 2, N], FP8, name=f"kkT2_{j}") for j in range(4)]
        qT2 = [kk_p.tile([D, 2, TOK_OWN], FP8, name=f"qT2_{j}") for j in range(4)]
        gTh = L0.enter_context(tc.tile_pool(name="gTh", bufs=1)) \
            .tile([P, H // 2, TOK_OWN], FP8, name="gTht")
        vpool = L0.enter_context(tc.tile_pool(name="vp", bufs=NTO))
        v2 = [vpool.tile([P, 2, VPAD], FP8, name=f"v2_{i}", tag="v2")
              for i in range(NTO)]
        ogT_p = L0.enter_context(tc.tile_pool(name="ogTp", bufs=1))
        ogT = ogT_p.tile([P, CT, TOK_OWN], FP8, name="ogT")
        wA = L0.enter_context(tc.tile_pool(name="wA", bufs=1))

        # scores PSUM on the right side
        psS = L0.enter_context(
            tc.tile_pool(name="psS", bufs=2, space="PSUM", side="right"))
        epool = L0.enter_context(tc.tile_pool(name="epool", bufs=20, side="right"))
        rpool = L0.enter_context(tc.tile_pool(name="rpool", bufs=4, side="right"))

        eps_sb = consts.tile([P, 1], FP32)
        nc.vector.memset(eps_sb[:], EPS)
        ident = consts.tile([P, P], BF16)
        make_identity(nc, ident[:])

        xm_pool = L0.enter_context(tc.tile_pool(name="xm", bufs=NTO))

        x_own = [xo_pool.tile([P, C], FP32, name=f"xown{t}", tag="xown")
                 for t in range(NTO)]

        # ---------- LN helper ----------
        def ln_group(tiles, mvtag, zT_all, t0, pspool, pstag, eng):
            """LN 4 tiles token-major, transpose to c-major fp8 planes of
            zT_all at column t*128.  eng: 'a' routes the PSUM->SBUF copies
            to ACT (idle during the front), else DVE."""
            g = len(tiles)
            mv = stat_pool.tile([P, g, 2], FP32, name=f"mv_{mvtag}", tag="mv")
            st6 = stat_pool.tile([P, 6], FP32, name=f"st6_{mvtag}", tag="st6")
            for i, xt in enumerate(tiles):
                nc.vector.bn_stats(st6[:], xt[:])
                nc.vector.bn_aggr(mv[:, i, :], st6[:])
            sd = stat_pool.tile([P, g], FP32, name=f"sd_{mvtag}", tag="sd")
            nc.scalar.activation(sd[:], mv[:, :, 1], Act.Sqrt, bias=eps_sb[:])
            rstd = stat_pool.tile([P, g], FP32, name=f"rstd_{mvtag}", tag="rstd")
            nc.vector.reciprocal_approx_fast(rstd[:], sd[:])
            for i, xt in enumerate(tiles):
                t = t0 + i
                zt = z_pool.tile([P, C], BF16, name=f"z_{mvtag}_{i}", tag="z")
                nc.vector.tensor_scalar(
                    zt[:], xt[:], mv[:, i, 0:1], rstd[:, i : i + 1],
                    Alu.subtract, Alu.mult,
                )
                pt = pspool.tile([P, CT, P], BF16, name=f"pt_{mvtag}", tag=pstag)
                for c in range(CT):
                    nc.tensor.transpose(
                        pt[:, c, :], zt[:, c * P : (c + 1) * P], ident[:])
                if eng == "a":
                    nc.scalar.copy(zT_all[:, :, t * P : (t + 1) * P], pt[:])
                else:
                    nc.vector.tensor_copy(
                        zT_all[:, :, t * P : (t + 1) * P], pt[:])

        # ---------- projection helpers (DR fp8) ----------
        def q_proj(m):
            pq = psC.tile([P, TOK_OWN], FP32, name=f"pq{m}", tag="pc")
            for i in range(2):
                lw = wqkv_sb[:, 2 * i : 2 * i + 2, m * P : (m + 1) * P]
                for ch in range(2):
                    nc.tensor.matmul(
                        pq[:, ch * 512 : (ch + 1) * 512], lw,
                        zxT[:, 2 * i : 2 * i + 2, ch * 512 : (ch + 1) * 512],
                        start=(i == 0), stop=(i == 1), perf_mode=DR,
                    )
            nc.scalar.copy(qT2[2 * (m // 2)][:, m % 2, :], pq[0:D, :])
            nc.scalar.copy(qT2[2 * (m // 2) + 1][:, m % 2, :],
                           pq[D : 2 * D, :])

        def kk_half(m, half):
            pc = psC.tile([P, TOK_OWN], FP32, name=f"pc{m}_{half}", tag="pc")
            base = half * TOK_OWN
            for i in range(2):
                lw = wqkv_sb[:, 2 * i : 2 * i + 2, C + m * P : C + (m + 1) * P]
                for ch in range(2):
                    sl = slice(base + ch * 512, base + (ch + 1) * 512)
                    nc.tensor.matmul(
                        pc[:, ch * 512 : (ch + 1) * 512], lw,
                        zxT[:, 2 * i : 2 * i + 2, sl],
                        start=(i == 0), stop=False, perf_mode=DR,
                    )
            for i in range(2):
                lw = ws_sb[:, 2 * i : 2 * i + 2, m * P : (m + 1) * P]
                for ch in range(2):
                    sl = slice(base + ch * 512, base + (ch + 1) * 512)
                    nc.tensor.matmul(
                        pc[:, ch * 512 : (ch + 1) * 512], lw,
                        zeT[:, 2 * i : 2 * i + 2, sl],
                        start=False, stop=(i == 1), perf_mode=DR,
                    )
            nc.scalar.copy(
                kkT2[2 * (m // 2)][:, m % 2, base : base + TOK_OWN], pc[0:D, :])
            nc.scalar.copy(
                kkT2[2 * (m // 2) + 1][:, m % 2, base : base + TOK_OWN],
                pc[D : 2 * D, :])

        def v_pair(tp):
            pv = psC.tile([P, TOK_OWN], FP32, name=f"pv{tp}", tag="pc")
            for i in range(2):
                for tt in range(2):
                    t = 2 * tp + tt
                    nc.tensor.matmul(
                        pv[:, tt * 512 : (tt + 1) * 512],
                        zxT[:, 2 * i : 2 * i + 2, t * P : (t + 1) * P],
                        wqkv_sb[:, 2 * i : 2 * i + 2, 2 * C : 3 * C],
                        start=(i == 0), stop=(i == 1), perf_mode=DR,
                    )
            nc.vector.memset(v2[tp][:, :, H * (D + 1) : VPAD], 0.0)
            va = v2[tp][:, :, 0 : H * (D + 1)].rearrange(
                "p two (h x) -> p two h x", x=D + 1)
            nc.vector.memset(va[:, :, :, D : D + 1], 1.0)
            for tt in range(2):
                nc.vector.tensor_copy(
                    va[:, tt, :, 0:D],
                    pv[:, tt * 512 : (tt + 1) * 512].rearrange(
                        "p (h d) -> p h d", d=D),
                )

        def gate_proj(m):
            pg = psC.tile([P, TOK_OWN], FP32, name=f"pg{m}", tag="pc")
            for i in range(2):
                lw = wgate_sb[:, 2 * i : 2 * i + 2, m * P : (m + 1) * P]
                for ch in range(2):
                    nc.tensor.matmul(
                        pg[:, ch * 512 : (ch + 1) * 512], lw,
                        zxT[:, 2 * i : 2 * i + 2, ch * 512 : (ch + 1) * 512],
                        start=(i == 0), stop=(i == 1), perf_mode=DR,
                    )
            nc.vector.tensor_copy(gTh[:, m, :], pg[:])

        # ---------- attention helpers ----------
        ep_tiles = {}  # (h, tp) -> Ep tile

        def scores_exp(h, t):
            jj, g2 = h // 2, h % 2
            sc = psS.tile([P, TOK_OWN], FP32, name=f"sc{h}_{t}", tag="sc")
            for ch in range(2):
                nc.tensor.matmul(
                    sc[:, ch * 512 : (ch + 1) * 512],
                    kkT2[jj][32 * g2 : 32 * g2 + 32, :, t * P : (t + 1) * P],
                    qT2[jj][32 * g2 : 32 * g2 + 32, :, ch * 512 : (ch + 1) * 512],
                    start=True, stop=True, perf_mode=DR,
                )
            tp = t // 2
            if t % 2 == 0:
                ep_tiles[(h, tp)] = epool.tile(
                    [P, 2, TOK_OWN], FP8, name=f"Ep{h}_{tp}", tag="E")
            nc.scalar.activation(
                ep_tiles[(h, tp)][:, t % 2, :], sc[:], Act.Exp, scale=EXP_SCALE)

        ps_att = {}

        def attnv(h, tp):
            if tp == 0:
                ps_att[h] = psO.tile([P, TOK_OWN], FP32, name=f"pso{h}", tag="po")
            ep = ep_tiles.pop((h, tp))
            for ch in range(2):
                nc.tensor.matmul(
                    ps_att[h][:, ch * 512 : (ch + 1) * 512],
                    v2[tp][:, :, h * (D + 1) : h * (D + 1) + P],
                    ep[:, :, ch * 512 : (ch + 1) * 512],
                    start=(tp == 0), stop=(tp == NTO - 1), perf_mode=DR,
                )
            if tp == NTO - 1:
                normalize(h)

        def normalize(h):
            ps_o = ps_att.pop(h)
            dn = rpool.tile([1, TOK_OWN], FP32, name="dn", tag="nrm")
            nc.vector.tensor_copy(dn[:], ps_o[D : D + 1, :])
            rdb1 = rpool.tile([1, TOK_OWN], FP32, name="rdb1", tag="nrm")
            nc.vector.reciprocal_approx_fast(rdb1[:], dn[:])
            dnb = rpool.tile([D, TOK_OWN], FP32, name="dnb", tag="nrm")
            nc.gpsimd.partition_broadcast(dnb[:], rdb1[:])
            t1 = rpool.tile([D, TOK_OWN], FP32, name="t1", tag="nrm")
            gpo = (h % 2) * D
            nc.vector.tensor_mul(t1[:], ps_o[0:D, :],
                                 gTh[gpo : gpo + D, h // 2, :])
            po = (h % 2) * D
            nc.gpsimd.tensor_mul(ogT[po : po + D, h // 2, :], t1[:], dnb[:])

        # ================= emission =================
        es_early = ExitStack()
        L2 = ExitStack()
        try:
            xr_pool = es_early.enter_context(tc.tile_pool(name="xrp", bufs=NTO))
            e_pool = es_early.enter_context(tc.tile_pool(name="ep", bufs=5))
            zeT_pool = es_early.enter_context(tc.tile_pool(name="zeTp", bufs=1))
            zeT = zeT_pool.tile([P, CT, N], FP8, name="zeT")
            wsp = es_early.enter_context(tc.tile_pool(name="wsp", bufs=1))

            psC = L2.enter_context(tc.tile_pool(name="psC", bufs=2, space="PSUM"))

            # ---- DMA in: weights + x/e half 0 first ----
            wqkv_sb = wA.tile([P, CT, 3 * C], FP8)
            nc.sync.dma_start(wqkv_sb[:], wqkv_d.rearrange("(k p) n -> p k n", p=P))
            ws_sb = wsp.tile([P, CT, C], FP8)
            nc.sync.dma_start(ws_sb[:], ws_d.rearrange("(k p) n -> p k n", p=P))

            x_all = list(x_own)
            e_all = []
            for t in range(NTO):
                nc.sync.dma_start(x_all[t][:], xb_r[t])
            for t in range(NTO):
                et = e_pool.tile([P, C], FP32, name=f"e{t}", tag="e")
                nc.sync.dma_start(et[:], eb_r[t])
                e_all.append(et)
            wgate_sb = wA.tile([P, CT, C], FP8)
            nc.sync.dma_start(wgate_sb[:], wgate_d.rearrange("(k p) n -> p k n", p=P))
            for t in range(NTO, NT):
                xt = xr_pool.tile([P, C], FP32, name=f"xr{t}", tag="xr")
                nc.sync.dma_start(xt[:], xb_r[t])
                x_all.append(xt)
            for t in range(NTO, NT):
                et = e_pool.tile([P, C], FP32, name=f"e{t}", tag="e")
                nc.sync.dma_start(et[:], eb_r[t])
                e_all.append(et)

            # ---- LN half 0 + early projections ----
            for g in range(2):
                ln_group(x_all[4 * g : 4 * g + 4], f"x{g}", zxT, 4 * g,
                         psC, "pc", "a")
                ln_group(e_all[4 * g : 4 * g + 4], f"e{g}", zeT, 4 * g,
                         psC, "pc", "a")
            q_proj(0)
            kk_half(0, 0)
            q_proj(1)
            kk_half(1, 0)

            # ---- prime: heads 0-3 x t 0-7 interleaved with the rest ----
            bg = deque()
            bg.append(lambda: ln_group(x_all[8:12], "x2", zxT, 8, psC, "pc", "a"))
            bg.append(lambda: ln_group(e_all[8:12], "e2", zeT, 8, psC, "pc", "a"))
            bg.append(lambda: ln_group(x_all[12:16], "x3", zxT, 12, psC, "pc", "a"))
            bg.append(lambda: ln_group(e_all[12:16], "e3", zeT, 12, psC, "pc", "a"))
            bg.append(lambda: kk_half(0, 1))
            bg.append(lambda: kk_half(1, 1))
            bg.append(lambda: v_pair(0))
            bg.append(lambda: v_pair(1))
            bg.append(lambda: v_pair(2))
            bg.append(lambda: v_pair(3))
            bg.append(lambda: q_proj(2))
            bg.append(lambda: q_proj(3))
            bg.append(lambda: kk_half(2, 0))
            bg.append(lambda: kk_half(3, 0))
            bg.append(lambda: kk_half(2, 1))
            bg.append(lambda: kk_half(3, 1))
            bg.append(lambda: v_pair(4))
            bg.append(lambda: v_pair(5))
            bg.append(lambda: v_pair(6))
            bg.append(lambda: v_pair(7))
            bg.append(lambda: gate_proj(0))
            bg.append(lambda: gate_proj(1))
            bg.append(lambda: gate_proj(2))
            bg.append(lambda: gate_proj(3))

            prime = [(h, t) for h in range(2) for t in range(8)]
            prime += [(h, t) for h in range(2, 4) for t in range(4)]
            for i, (h, t) in enumerate(prime):
                if bg:
                    bg.popleft()()
                scores_exp(h, t)
            while bg:
                bg.popleft()()

            es_early.close()  # xr, e tiles, zeT, ws freed
        finally:
            L2.close()  # psC banks freed before psO opens

        # MLP weights: DMA early so the tail never waits on HBM
        wE = L0.enter_context(tc.tile_pool(name="wE", bufs=1))
        wproj_sb = wE.tile([P, CT, C], FP8)
        nc.sync.dma_start(wproj_sb[:],
                          wproj_d.rearrange("(k p) n -> p k n", p=P))
        wfc1_sb = wE.tile([P, CT, DFF], FP8)
        nc.sync.dma_start(wfc1_sb[:],
                          wfc1_d.rearrange("(k p) n -> p k n", p=P))
        wfc2_sb = wE.tile([P, DFT, C], FP8)
        nc.sync.dma_start(wfc2_sb[:],
                          wfc2_d.rearrange("(k p) n -> p k n", p=P))

        # ---- steady: remaining exps + head-sequential attnv ----
        with tc.tile_pool(name="psO", bufs=2, space="PSUM") as psO:
            steady = [(h, t) for h in range(2) for t in range(8, 16)]
            steady += [(h, t) for h in range(2, 4) for t in range(4, 16)]
            steady += [(h, t) for h in range(4, 8) for t in range(16)]
            done_exp = {(h, tp): True for h in range(2) for tp in range(4)}
            done_exp.update({(h, tp): True for h in range(2, 4) for tp in range(2)})

            # pop attnv strictly in (head, tp) lexicographic order
            attnv_order = [(h, tp) for h in range(H) for tp in range(NTO)]
            cursor = 0

            def pump(budget):
                nonlocal cursor
                n = 0
                while n < budget and cursor < len(attnv_order):
                    item = attnv_order[cursor]
                    if not done_exp.get(item):
                        break
                    attnv(*item)
                    cursor += 1
                    n += 1

            lag = 12  # exp chunks of headroom before attnv consumes
            for i, (h, t) in enumerate(steady):
                # emit ready attnv work BEFORE the (possibly psS-stalled)
                # scores: the PE executes its queue in order, so a stalled
                # matmul would block ready work queued behind it
                if i >= lag:
                    pump(1)
                scores_exp(h, t)
                if t % 2 == 1:
                    done_exp[(h, t // 2)] = True
            pump(len(attnv_order))

            # ---------- proj + residual + LN3 + MLP ----------
            with (
                tc.tile_pool(name="z3Tp", bufs=1) as z3T_pool,
                tc.tile_pool(name="hTp", bufs=1) as hT_pool,
                tc.tile_pool(name="opool", bufs=4) as opool,
            ):
                xm = []

                def proj_pair(tpair):
                    pp = psO.tile([P, TOK_OWN], FP32, name=f"pp{tpair}", tag="po")
                    for tt in range(2):
                        t = 2 * tpair + tt
                        for i in range(2):
                            nc.tensor.matmul(
                                pp[:, tt * 512 : (tt + 1) * 512],
                                ogT[:, 2 * i : 2 * i + 2, t * P : (t + 1) * P],
                                wproj_sb[:, 2 * i : 2 * i + 2, :],
                                start=(i == 0), stop=(i == 1), perf_mode=DR,
                            )
                    for tt in range(2):
                        t = 2 * tpair + tt
                        xmt = xm_pool.tile([P, C], FP32, name=f"xm{t}", tag="xm")
                        nc.vector.scalar_tensor_tensor(
                            xmt[:], pp[:, tt * 512 : (tt + 1) * 512],
                            PROJ_COMP, x_own[t][:],
                            Alu.mult, Alu.add,
                        )
                        xm.append(xmt)

                z3T = z3T_pool.tile([P, CT, TOK_OWN], FP8, name="z3T")
                hT = hT_pool.tile([P, DFT, TOK_OWN], FP8, name="hT")
                # all projections + both LN3 groups up front: the LN3 DVE
                # stats/applies then overlap the fc1/fc2 PE stream
                proj_pair(0)
                proj_pair(1)
                ln_group(xm[0:4], "x30", z3T, 0, psS, "sc", "v")
                proj_pair(2)
                proj_pair(3)
                ln_group(xm[4:8], "x31", z3T, 4, psS, "sc", "v")
                for ch in range(2):
                    sl = slice(ch * 512, (ch + 1) * 512)
                    pf2w = [psO.tile([P, TOK_OWN], FP32, name=f"pf2w{ch}{i}",
                                     tag="po") for i in range(2)]
                    for mm in range(DFT):
                        pf = psS.tile([P, 512], FP32, name=f"pf{ch}", tag="sc")
                        for i in range(2):
                            nc.tensor.matmul(
                                pf[:],
                                wfc1_sb[:, 2 * i : 2 * i + 2,
                                        mm * P : (mm + 1) * P],
                                z3T[:, 2 * i : 2 * i + 2, sl],
                                start=(i == 0), stop=(i == 1), perf_mode=DR,
                            )
                        nc.scalar.activation(hT[:, mm, sl], pf[:], Act.Gelu,
                                             scale=GELU_SCALE)
                        if mm % 2 == 1:
                            i = mm // 2
                            for tt in range(4):
                                t = 4 * ch + tt
                                nc.tensor.matmul(
                                    pf2w[tt // 2][:, (tt % 2) * 512
                                                  : (tt % 2 + 1) * 512],
                                    hT[:, 2 * i : 2 * i + 2,
                                       t * P : (t + 1) * P],
                                    wfc2_sb[:, 2 * i : 2 * i + 2, :],
                                    start=(i == 0), stop=(i == DFT // 2 - 1),
                                    perf_mode=DR,
                                )
                    for tt in range(4):
                        t = 4 * ch + tt
                        ot = opool.tile([P, C], FP32, name="ot", tag="ot")
                        nc.vector.scalar_tensor_tensor(
                            ot[:], pf2w[tt // 2][:, (tt % 2) * 512
                                                 : (tt % 2 + 1) * 512],
                            FC2_COMP, xm[t][:],
                            Alu.mult, Alu.add,
                        )
                        nc.sync.dma_start(out_r[t], ot[:])


def _preprocess(inputs):
    """Fold LN affine + attention scale + fp8 scaling into weights, apply
    the q/k column permutation (host-side, weight-only)."""
    f32 = np.float32
    ln1_w, ln1_b = f32(inputs["ln1_w"]), f32(inputs["ln1_b"])
    ln2_b = f32(inputs["ln2_b"])
    ln3_b = f32(inputs["ln3_b"])
    ln2_w = f32(inputs["ln2_w"])
    ln3_w = f32(inputs["ln3_w"])
    w_qkv = f32(inputs["w_qkv"]).copy()
    w_s = f32(inputs["w_s"])
    w_gate = f32(inputs["w_gate"])
    w_proj = f32(inputs["w_proj"])
    w_fc1 = f32(inputs["w_fc1"])
    w_fc2 = f32(inputs["w_fc2"])

    scale = D ** -0.5
    wqkv_eff = ln1_w[:, None] * w_qkv
    wqkv_eff[:, 0:C] *= scale * SQ
    wqkv_eff[:, C : 2 * C] *= SK
    wqkv_eff[:, 2 * C : 3 * C] *= SV
    b_qkv = ln1_b @ w_qkv
    ws_eff = ln2_w[:, None] * w_s * SK
    b_s = ln2_b @ w_s
    wgate_eff = ln1_w[:, None] * w_gate * SG
    b_gate = ln1_b @ w_gate
    wfc1_eff = ln3_w[:, None] * w_fc1 * SF1
    b_fc1 = ln3_b @ w_fc1 + f32(inputs["b_fc1"])
    wfc2_eff = w_fc2 * SF2

    for name, bias in [
        ("b_qkv", b_qkv), ("b_s", b_s), ("b_gate", b_gate), ("b_fc1", b_fc1),
        ("b_proj", f32(inputs["b_proj"])), ("b_fc2", f32(inputs["b_fc2"])),
    ]:
        assert np.all(bias == 0.0), f"nonzero bias {name} unsupported"

    perm = _qk_perm()
    wqkv_eff[:, 0:C] = wqkv_eff[:, perm]
    wqkv_eff[:, C : 2 * C] = wqkv_eff[:, C + perm]
    ws_eff = ws_eff[:, perm]

    f8 = ml_dtypes.float8_e4m3fn
    return {
        "wqkv": np.ascontiguousarray(wqkv_eff, dtype=f8),
        "ws": np.ascontiguousarray(ws_eff, dtype=f8),
        "wgate": np.ascontiguousarray(wgate_eff, dtype=f8),
        "wproj": np.ascontiguousarray(w_proj * SP, dtype=f8),
        "wfc1": np.ascontiguousarray(wfc1_eff, dtype=f8),
        "wfc2": np.ascontiguousarray(wfc2_eff, dtype=f8),
    }


def kernel(**inputs):
    from concourse import bass_utils

    _enable_ldw_opt()
    if "nc" not in _CACHE:
        _CACHE["nc"] = _build_nc()
    nc = _CACHE["nc"]

    w = _preprocess(inputs)
    x = np.asarray(inputs["x"], dtype=np.float32)
    e = np.asarray(inputs["e"], dtype=np.float32)

    in_maps = []
    for c in range(N_CORES):
        b, half = c // 2, c % 2
        if half == 0:
            xb, eb = x[b], e[b]
        else:
            xb = np.concatenate([x[b, TOK_OWN:], x[b, :TOK_OWN]], axis=0)
            eb = np.concatenate([e[b, TOK_OWN:], e[b, :TOK_OWN]], axis=0)
        in_maps.append({
            "xb": np.ascontiguousarray(xb),
            "eb": np.ascontiguousarray(eb),
            **w,
        })

    res = bass_utils.run_bass_kernel_spmd(
        nc, in_maps, core_ids=list(range(N_CORES)),
        trace=_CACHE.get("trace", False),
    )
    _CACHE["last_result"] = res

    out = np.empty((B, N, C), dtype=np.float32)
    for c in range(N_CORES):
        b, half = c // 2, c % 2
        out[b, half * TOK_OWN : (half + 1) * TOK_OWN] = res.results[c]["out"]
    return out
